# revision 43
# baseline (speedup 1.0000x reference)
"""MultiHeadAttention Bass/Tile kernel for Trainium2, 8 NeuronCores.

Sharding: (batch, query-half) -> 8 cores, zero collectives.
  core c: batch b = c//2, query rows qh = c%2 (1024 rows each).

Per-core dataflow (all transposes done on HOST, so every matmul operand
loads naturally with its contraction dim on partitions):
  P0: Q^T[e,q], K^T[e,s] f32r; V[s,(h,65)] bf16 (col 64 = ones -> denominator)
  P1: per head h, per k-strip: S^T[k,q] = K_h^T.T @ Q_h^T (f32r, PSUM)
      -> exp(S^T/8) bf16 (ACT, evacuates PSUM) -> * (1-mask^T) bf16 (DVE)
      -> PV accumulate out^T[65,q] over 16 k-strips (bf16 matmul)
  P2: 1/den via ACT ln+exp, DMA-broadcast over partitions, normalize out^T
      in place, out-projection y[q,eo] = sum_e OA[e,q] woT[e,eo] (f32r).
"""

import os
import sys

for _p in ("/opt/trn_rl_repo", "/root/.axon_site/_ro/trn_rl_repo"):
    if os.path.isdir(_p) and _p not in sys.path:
        sys.path.insert(0, _p)

from contextlib import ExitStack

import numpy as np

import concourse.tile as tile
from concourse import bacc, mybir
from concourse.bass_utils import run_bass_kernel_spmd

B, S, D = 4, 2048, 1024
H, HD = 16, 64
Q = S // 2  # per-core query rows
NCORES = 8

F32 = mybir.dt.float32
F32R = mybir.dt.float32r
BF16 = mybir.dt.bfloat16
I32 = mybir.dt.int32

_NC_CACHE = {}


def _r(ap):
    return ap.bitcast(F32R)


def _build_kernel(tc, t_in, t_out, phases="all"):
    nc = tc.nc
    qT, kT, vT, mT = t_in["qT"], t_in["kT"], t_in["vT"], t_in["mT"]
    wqT, wkT, wvT, woT = t_in["wqT"], t_in["wkT"], t_in["wvT"], t_in["woT"]
    y = t_out["y"]

    qT3 = qT[:, :].rearrange("(po pi) q -> pi po q", pi=128)  # [128, 8, Q]
    kT3 = kT[:, :].rearrange("(po pi) s -> pi po s", pi=128)
    vT3 = vT[:, :].rearrange("(po pi) s -> pi po s", pi=128)
    wq3 = wqT[:, :].rearrange("(po pi) e -> pi po e", pi=128)
    wk3 = wkT[:, :].rearrange("(po pi) e -> pi po e", pi=128)
    wv3 = wvT[:, :].rearrange("(po pi) e -> pi po e", pi=128)
    wo3 = woT[:, :].rearrange("(po pi) e -> pi po e", pi=128)

    with ExitStack() as ctx:
        dram = ctx.enter_context(tc.tile_pool(name="dram", bufs=1, space="DRAM"))
        den_dram = dram.tile([H, Q], F32)  # per-head softmax denominators
        denr_dram = dram.tile([H, Q], F32)  # their reciprocals
        oa_dram = dram.tile([D, Q], F32)  # unnormalized out_attn^T [e, q]

        pctx = ctx.enter_context(ExitStack())
        persist = pctx.enter_context(tc.tile_pool(name="persist", bufs=1))
        QT = persist.tile([128, 8, Q], F32R)  # [e%128, e//128, q]
        kTb = persist.tile([128, 8, S], BF16)  # key^T input, bf16 (K-proj in P1)
        WKb = persist.tile([128, 8, D], BF16)  # wk^T, bf16
        V = persist.tile([128, 16, H, HD + 1], BF16)  # [s%128, s//128, h, d|one]

        # ---- P0: projections ----
        with (
            tc.tile_pool(name="wpool", bufs=2) as wpool,
            tc.tile_pool(name="apool", bufs=3) as apool,
            tc.tile_pool(name="pj", bufs=4, space="PSUM") as pj,
            tc.tile_pool(name="pjv", bufs=2, space="PSUM") as pjv,
        ):
            # Q^T[e, q] = sum_d wqT[d, e] * qT[d, q]
            WQ = wpool.tile([128, 8, D], F32R, tag="w")
            nc.sync.dma_start(out=WQ, in_=_r(wq3))
            for qn in range(Q // 256):
                qa = apool.tile([128, 8, 256], F32R, tag="a")
                nc.sync.dma_start(out=qa, in_=_r(qT3[:, :, qn * 256 : (qn + 1) * 256]))
                for eo in range(8):
                    ps = pj.tile([128, 256], F32)
                    for dc in range(8):
                        nc.tensor.matmul(
                            ps,
                            WQ[:, dc, eo * 128 : (eo + 1) * 128],
                            qa[:, dc, :],
                            start=(dc == 0),
                            stop=(dc == 7),
                        )
                    nc.vector.tensor_copy(
                        out=QT[:, eo, qn * 256 : (qn + 1) * 256], in_=ps
                    )


            # stage wk^T (bf16) for just-in-time K-proj inside P1; interleave
            # key^T staging DMAs with the V projection so PE never waits
            WV = wpool.tile([128, 8, D], F32R, tag="w")
            nc.sync.dma_start(out=WV, in_=_r(wv3))
            nc.vector.memset(V[:, :, :, 64:65], 1.0)
            for sn in range(S // 256):
                va = apool.tile([128, 8, 256], F32R, tag="a")
                nc.sync.dma_start(out=va, in_=_r(vT3[:, :, sn * 256 : (sn + 1) * 256]))
                ka = apool.tile([128, 8, 256], F32R, tag="a")
                nc.sync.dma_start(out=ka, in_=_r(kT3[:, :, sn * 256 : (sn + 1) * 256]))
                nc.vector.tensor_copy(
                    out=kTb[:, :, sn * 256 : (sn + 1) * 256], in_=ka.bitcast(F32)
                )
                for stl in range(2):
                    st = sn * 2 + stl
                    ps = pjv.tile([128, 1024], F32)
                    for dc in range(8):
                        for en in range(2):
                            nc.tensor.matmul(
                                ps[:, en * 512 : (en + 1) * 512],
                                va[:, dc, stl * 128 : (stl + 1) * 128],
                                WV[:, dc, en * 512 : (en + 1) * 512],
                                start=(dc == 0),
                                stop=(dc == 7),
                            )
                    nc.vector.tensor_copy(
                        out=V[:, st, :, 0:64],
                        in_=ps[:].rearrange("p (h d) -> p h d", h=H),
                    )

            wkf = wpool.tile([128, 8, D], F32R, tag="w")
            nc.sync.dma_start(out=wkf, in_=_r(wk3))
            nc.vector.tensor_copy(out=WKb, in_=wkf.bitcast(F32))

        if phases == "p0":
            return
        # ---- mask: MB[k%128, k//128, q] = 1 - mask^T, bf16 ----
        with ExitStack() as mctx:
            mb_pool = mctx.enter_context(tc.tile_pool(name="mb", bufs=1))
            MB = mb_pool.tile([128, 16, Q], BF16)
            with tc.tile_pool(name="mint", bufs=3) as mint:
                for ks in range(16):
                    mi = mint.tile([128, Q], I32)
                    nc.sync.dma_start(out=mi, in_=mT[ks * 128 : (ks + 1) * 128, :])
                    nc.vector.tensor_scalar(
                        out=MB[:, ks, :],
                        in0=mi,
                        scalar1=-1.0,
                        scalar2=1.0,
                        op0=mybir.AluOpType.mult,
                        op1=mybir.AluOpType.add,
                    )

            if phases == "p0mask":
                return
            # ---- P1: per e-chunk: K-proj (just-in-time) + its 2 heads ----
            with (
                tc.tile_pool(name="ebuf", bufs=3) as ebuf,
                tc.tile_pool(name="pbuf", bufs=3) as pbuf,
                tc.tile_pool(name="dtmp", bufs=2) as dtmp,
                tc.tile_pool(name="oab", bufs=3) as oab,
                tc.tile_pool(name="kte", bufs=3) as kte,
                tc.tile_pool(name="psS", bufs=2, space="PSUM") as psS,
                tc.tile_pool(name="pjK", bufs=2, space="PSUM") as pjK,
                tc.tile_pool(name="psPV", bufs=1, space="PSUM") as psPV,
            ):
                for eo in range(8):
                    # K^T[e-chunk eo, s] = sum_d wkT[d, e] kT[d, s]  (bf16 mm)
                    KTe = kte.tile([128, S], F32R, tag="kte")
                    for sn in range(S // 512):
                        kps = pjK.tile([128, 512], F32)
                        for dc in range(8):
                            nc.tensor.matmul(
                                kps,
                                WKb[:, dc, eo * 128 : (eo + 1) * 128],
                                kTb[:, dc, sn * 512 : (sn + 1) * 512],
                                start=(dc == 0),
                                stop=(dc == 7),
                            )
                        nc.vector.tensor_copy(
                            out=KTe[:, sn * 512 : (sn + 1) * 512], in_=kps
                        )
                    for h in (2 * eo, 2 * eo + 1):
                        hp = 64 * (h % 2)
                        pv = psPV.tile([HD + 1, Q], F32)
                        for ks in range(16):
                            sps = psS.tile([128, Q], F32)
                            lhsT = KTe[hp : hp + 64, ks * 128 : (ks + 1) * 128]
                            for qn in range(Q // 512):
                                nc.tensor.matmul(
                                    sps[:, qn * 512 : (qn + 1) * 512],
                                    lhsT,
                                    QT[hp : hp + 64, eo, qn * 512 : (qn + 1) * 512],
                                    start=True,
                                    stop=True,
                                )
                            eb = ebuf.tile([128, Q], BF16)
                            nc.scalar.activation(
                                out=eb,
                                in_=sps,
                                func=mybir.ActivationFunctionType.Exp,
                                scale=0.125,
                            )
                            pb = pbuf.tile([128, Q], BF16)
                            nc.vector.tensor_tensor(
                                out=pb,
                                in0=eb,
                                in1=MB[:, ks, :],
                                op=mybir.AluOpType.mult,
                            )
                            for qn in range(Q // 512):
                                nc.tensor.matmul(
                                    pv[:, qn * 512 : (qn + 1) * 512],
                                    V[:, ks, h, :],
                                    pb[:, qn * 512 : (qn + 1) * 512],
                                    start=(ks == 0),
                                    stop=(ks == 15),
                                )
                        ob = oab.tile([64, Q], F32)
                        nc.vector.tensor_copy(out=ob, in_=pv[0:64, :])
                        nc.sync.dma_start(
                            out=oa_dram[h * 64 : (h + 1) * 64, :], in_=ob
                        )
                        dt = dtmp.tile([1, Q], F32)
                        nc.vector.tensor_copy(out=dt, in_=pv[64:65, :])
                        nc.sync.dma_start(out=den_dram[h : h + 1, :], in_=dt)

        if phases == "p0p1":
            return
        # QT/KT/V dead after P1 -- release their SBUF before P2
        pctx.close()

        # ---- P2: normalize + output projection ----
        with (
            tc.tile_pool(name="p2", bufs=1) as p2,
            tc.tile_pool(name="rb", bufs=2) as rb,
            tc.tile_pool(name="ybuf", bufs=2) as ybuf,
            tc.tile_pool(name="psY", bufs=4, space="PSUM") as psY,
        ):
            WO = p2.tile([128, 8, D], F32R)
            nc.sync.dma_start(out=WO, in_=_r(wo3))
            OA = p2.tile([128, 8, Q], F32R)
            oa_dram3 = oa_dram[:, :].rearrange("(jo ji) q -> ji jo q", ji=128)
            for j in range(8):
                nc.sync.dma_start(out=OA[:, j, :], in_=_r(oa_dram3[:, j, :]))

            # 1/den via ACT: exp(-ln(den)) (same table set as Exp)
            den_sb = p2.tile([H, Q], F32)
            nc.sync.dma_start(out=den_sb, in_=den_dram[:, :])
            den_r = p2.tile([H, Q], F32)
            nc.scalar.activation(
                out=den_r, in_=den_sb, func=mybir.ActivationFunctionType.Ln
            )
            nc.scalar.activation(
                out=den_r,
                in_=den_r,
                func=mybir.ActivationFunctionType.Exp,
                scale=-1.0,
            )
            nc.sync.dma_start(out=denr_dram[:, :], in_=den_r)

            for j in range(8):
                # R[p, q]: rows 0:64 = 1/den[2j], rows 64:128 = 1/den[2j+1]
                R = rb.tile([128, Q], F32)
                for half in range(2):
                    nc.sync.dma_start(
                        out=R[half * 64 : (half + 1) * 64, :],
                        in_=denr_dram[2 * j + half : 2 * j + half + 1, :].to_broadcast(
                            (64, Q)
                        ),
                    )
                nc.vector.tensor_tensor(
                    out=OA[:, j, :],
                    in0=OA[:, j, :].bitcast(F32),
                    in1=R,
                    op=mybir.AluOpType.mult,
                )

            # y[q, eo] = sum_e OA[e, q] * woT[e, eo]
            for qt in range(Q // 128):
                ps = psY.tile([128, D], F32)
                for j in range(8):
                    for en in range(2):
                        nc.tensor.matmul(
                            ps[:, en * 512 : (en + 1) * 512],
                            OA[:, j, qt * 128 : (qt + 1) * 128],
                            WO[:, j, en * 512 : (en + 1) * 512],
                            start=(j == 0),
                            stop=(j == 7),
                        )
                yb = ybuf.tile([128, D], F32)
                nc.vector.tensor_copy(out=yb, in_=ps)
                nc.sync.dma_start(out=y[qt * 128 : (qt + 1) * 128, :], in_=yb)


def _get_nc(phases="all"):
    if phases in _NC_CACHE:
        return _NC_CACHE[phases]
    nc = bacc.Bacc("TRN2", target_bir_lowering=False)
    t_in = {
        "qT": nc.dram_tensor("qT", [D, Q], F32, kind="ExternalInput"),
        "kT": nc.dram_tensor("kT", [D, S], F32, kind="ExternalInput"),
        "vT": nc.dram_tensor("vT", [D, S], F32, kind="ExternalInput"),
        "mT": nc.dram_tensor("mT", [S, Q], I32, kind="ExternalInput"),
        "wqT": nc.dram_tensor("wqT", [D, D], F32, kind="ExternalInput"),
        "wkT": nc.dram_tensor("wkT", [D, D], F32, kind="ExternalInput"),
        "wvT": nc.dram_tensor("wvT", [D, D], F32, kind="ExternalInput"),
        "woT": nc.dram_tensor("woT", [D, D], F32, kind="ExternalInput"),
    }
    t_out = {"y": nc.dram_tensor("y", [Q, D], F32, kind="ExternalOutput")}
    with tile.TileContext(nc) as tc:
        _build_kernel(tc, t_in, t_out, phases=phases)
    nc.compile()
    _NC_CACHE[phases] = nc
    return nc


def _in_maps(inputs):
    q = np.asarray(inputs["query"], np.float32)
    k = np.asarray(inputs["key"], np.float32)
    v = np.asarray(inputs["value"], np.float32)
    mask = np.asarray(inputs["mask"], np.int32)
    wqT = np.ascontiguousarray(np.asarray(inputs["wq"], np.float32).T)
    wkT = np.ascontiguousarray(np.asarray(inputs["wk"], np.float32).T)
    wvT = np.ascontiguousarray(np.asarray(inputs["wv"], np.float32).T)
    woT = np.ascontiguousarray(np.asarray(inputs["w_out"], np.float32).T)
    maps = []
    for c in range(NCORES):
        b, qh = c // 2, c % 2
        sl = slice(qh * Q, (qh + 1) * Q)
        maps.append(
            {
                "qT": np.ascontiguousarray(q[b].T[:, sl]),
                "kT": np.ascontiguousarray(k[b].T),
                "vT": np.ascontiguousarray(v[b].T),
                "mT": np.ascontiguousarray(mask[b].T[:, sl]),
                "wqT": wqT,
                "wkT": wkT,
                "wvT": wvT,
                "woT": woT,
            }
        )
    return maps


def _gather(res):
    outs = [res.results[c]["y"] for c in range(NCORES)]
    return np.stack(
        [np.concatenate([outs[2 * b], outs[2 * b + 1]], axis=0) for b in range(B)]
    )


def kernel(**inputs) -> np.ndarray:
    nc = _get_nc()
    res = run_bass_kernel_spmd(nc, _in_maps(inputs), core_ids=list(range(NCORES)))
    return _gather(res)


def kernel_traced(**inputs):
    """Like kernel() but with NTFF tracing; returns (output, BassKernelResults)."""
    nc = _get_nc()
    res = run_bass_kernel_spmd(
        nc, _in_maps(inputs), core_ids=list(range(NCORES)), trace=True
    )
    return _gather(res), res


# revision 44
# speedup vs baseline: 1.0087x; 1.0087x over previous
"""MultiHeadAttention Bass/Tile kernel for Trainium2, 8 NeuronCores.

Sharding: (batch, query-half) -> 8 cores, zero collectives.
  core c: batch b = c//2, query rows qh = c%2 (1024 rows each).

Per-core dataflow (all transposes done on HOST, so every matmul operand
loads naturally with its contraction dim on partitions):
  P0: Q^T[e,q], K^T[e,s] f32r; V[s,(h,65)] bf16 (col 64 = ones -> denominator)
  P1: per head h, per k-strip: S^T[k,q] = K_h^T.T @ Q_h^T (f32r, PSUM)
      -> exp(S^T/8) bf16 (ACT, evacuates PSUM) -> * (1-mask^T) bf16 (DVE)
      -> PV accumulate out^T[65,q] over 16 k-strips (bf16 matmul)
  P2: 1/den via ACT ln+exp, DMA-broadcast over partitions, normalize out^T
      in place, out-projection y[q,eo] = sum_e OA[e,q] woT[e,eo] (f32r).
"""

import os
import sys

for _p in ("/opt/trn_rl_repo", "/root/.axon_site/_ro/trn_rl_repo"):
    if os.path.isdir(_p) and _p not in sys.path:
        sys.path.insert(0, _p)

from contextlib import ExitStack

import numpy as np

import concourse.tile as tile
from concourse import bacc, mybir
from concourse.bass_utils import run_bass_kernel_spmd

B, S, D = 4, 2048, 1024
H, HD = 16, 64
Q = S // 2  # per-core query rows
NCORES = 8

F32 = mybir.dt.float32
F32R = mybir.dt.float32r
BF16 = mybir.dt.bfloat16
I32 = mybir.dt.int32

_NC_CACHE = {}


def _r(ap):
    return ap.bitcast(F32R)


def _build_kernel(tc, t_in, t_out, phases="all"):
    nc = tc.nc
    qT, kT, vT, mT = t_in["qT"], t_in["kT"], t_in["vT"], t_in["mT"]
    wqT, wkT, wvT, woT = t_in["wqT"], t_in["wkT"], t_in["wvT"], t_in["woT"]
    y = t_out["y"]

    qT3 = qT[:, :].rearrange("(po pi) q -> pi po q", pi=128)  # [128, 8, Q]
    kT3 = kT[:, :].rearrange("(po pi) s -> pi po s", pi=128)
    vT3 = vT[:, :].rearrange("(po pi) s -> pi po s", pi=128)
    wq3 = wqT[:, :].rearrange("(po pi) e -> pi po e", pi=128)
    wk3 = wkT[:, :].rearrange("(po pi) e -> pi po e", pi=128)
    wv3 = wvT[:, :].rearrange("(po pi) e -> pi po e", pi=128)
    wo3 = woT[:, :].rearrange("(po pi) e -> pi po e", pi=128)

    with ExitStack() as ctx:
        dram = ctx.enter_context(tc.tile_pool(name="dram", bufs=1, space="DRAM"))
        den_dram = dram.tile([H, Q], F32)  # per-head softmax denominators
        denr_dram = dram.tile([H, Q], F32)  # their reciprocals
        oa_dram = dram.tile([D, Q], F32)  # unnormalized out_attn^T [e, q]

        pctx = ctx.enter_context(ExitStack())
        persist = pctx.enter_context(tc.tile_pool(name="persist", bufs=1))
        QT = persist.tile([128, 8, Q], F32R)  # [e%128, e//128, q]
        kTb = persist.tile([128, 8, S], BF16)  # key^T input, bf16 (K-proj in P1)
        WKb = persist.tile([128, 8, D], BF16)  # wk^T, bf16
        V = persist.tile([128, 16, H, HD + 1], BF16)  # [s%128, s//128, h, d|one]

        # ---- P0: projections ----
        with (
            tc.tile_pool(name="wpool", bufs=2) as wpool,
            tc.tile_pool(name="apool", bufs=3) as apool,
            tc.tile_pool(name="pj", bufs=2, space="PSUM") as pj,
            tc.tile_pool(name="pjv", bufs=3, space="PSUM") as pjv,
        ):
            # Q^T[e, q] = sum_d wqT[d, e] * qT[d, q]
            WQ = wpool.tile([128, 8, D], F32R, tag="w")
            nc.sync.dma_start(out=WQ, in_=_r(wq3))
            for qn in range(Q // 256):
                qa = apool.tile([128, 8, 256], F32R, tag="a")
                nc.sync.dma_start(out=qa, in_=_r(qT3[:, :, qn * 256 : (qn + 1) * 256]))
                for eo in range(8):
                    ps = pj.tile([128, 256], F32)
                    for dc in range(8):
                        nc.tensor.matmul(
                            ps,
                            WQ[:, dc, eo * 128 : (eo + 1) * 128],
                            qa[:, dc, :],
                            start=(dc == 0),
                            stop=(dc == 7),
                        )
                    nc.vector.tensor_copy(
                        out=QT[:, eo, qn * 256 : (qn + 1) * 256], in_=ps
                    )


            # stage wk^T (bf16) for just-in-time K-proj inside P1; interleave
            # key^T staging DMAs with the V projection so PE never waits
            WV = wpool.tile([128, 8, D], F32R, tag="w")
            nc.sync.dma_start(out=WV, in_=_r(wv3))
            nc.vector.memset(V[:, :, :, 64:65], 1.0)
            for sn in range(S // 256):
                va = apool.tile([128, 8, 256], F32R, tag="a")
                nc.sync.dma_start(out=va, in_=_r(vT3[:, :, sn * 256 : (sn + 1) * 256]))
                ka = apool.tile([128, 8, 256], F32R, tag="a")
                nc.sync.dma_start(out=ka, in_=_r(kT3[:, :, sn * 256 : (sn + 1) * 256]))
                nc.vector.tensor_copy(
                    out=kTb[:, :, sn * 256 : (sn + 1) * 256], in_=ka.bitcast(F32)
                )
                for stl in range(2):
                    st = sn * 2 + stl
                    ps = pjv.tile([128, 1024], F32)
                    for dc in range(8):
                        for en in range(2):
                            nc.tensor.matmul(
                                ps[:, en * 512 : (en + 1) * 512],
                                va[:, dc, stl * 128 : (stl + 1) * 128],
                                WV[:, dc, en * 512 : (en + 1) * 512],
                                start=(dc == 0),
                                stop=(dc == 7),
                            )
                    nc.vector.tensor_copy(
                        out=V[:, st, :, 0:64],
                        in_=ps[:].rearrange("p (h d) -> p h d", h=H),
                    )

            wkf = wpool.tile([128, 8, D], F32R, tag="w")
            nc.sync.dma_start(out=wkf, in_=_r(wk3))
            nc.vector.tensor_copy(out=WKb, in_=wkf.bitcast(F32))

        if phases == "p0":
            return
        # ---- mask: MB[k%128, k//128, q] = 1 - mask^T, bf16 ----
        with ExitStack() as mctx:
            mb_pool = mctx.enter_context(tc.tile_pool(name="mb", bufs=1))
            MB = mb_pool.tile([128, 16, Q], BF16)
            with tc.tile_pool(name="mint", bufs=3) as mint:
                for ks in range(16):
                    mi = mint.tile([128, Q], I32)
                    nc.sync.dma_start(out=mi, in_=mT[ks * 128 : (ks + 1) * 128, :])
                    nc.vector.tensor_scalar(
                        out=MB[:, ks, :],
                        in0=mi,
                        scalar1=-1.0,
                        scalar2=1.0,
                        op0=mybir.AluOpType.mult,
                        op1=mybir.AluOpType.add,
                    )

            if phases == "p0mask":
                return
            # ---- P1: per e-chunk: K-proj (just-in-time) + its 2 heads ----
            with (
                tc.tile_pool(name="ebuf", bufs=3) as ebuf,
                tc.tile_pool(name="pbuf", bufs=3) as pbuf,
                tc.tile_pool(name="dtmp", bufs=2) as dtmp,
                tc.tile_pool(name="oab", bufs=3) as oab,
                tc.tile_pool(name="kte", bufs=3) as kte,
                tc.tile_pool(name="psS", bufs=2, space="PSUM") as psS,
                tc.tile_pool(name="pjK", bufs=2, space="PSUM") as pjK,
                tc.tile_pool(name="psPV", bufs=1, space="PSUM") as psPV,
            ):
                for eo in range(8):
                    # K^T[e-chunk eo, s] = sum_d wkT[d, e] kT[d, s]  (bf16 mm)
                    KTe = kte.tile([128, S], F32R, tag="kte")
                    for sn in range(S // 512):
                        kps = pjK.tile([128, 512], F32)
                        for dc in range(8):
                            nc.tensor.matmul(
                                kps,
                                WKb[:, dc, eo * 128 : (eo + 1) * 128],
                                kTb[:, dc, sn * 512 : (sn + 1) * 512],
                                start=(dc == 0),
                                stop=(dc == 7),
                            )
                        nc.vector.tensor_copy(
                            out=KTe[:, sn * 512 : (sn + 1) * 512], in_=kps
                        )
                    for h in (2 * eo, 2 * eo + 1):
                        hp = 64 * (h % 2)
                        pv = psPV.tile([HD + 1, Q], F32)
                        for ks in range(16):
                            sps = psS.tile([128, Q], F32)
                            lhsT = KTe[hp : hp + 64, ks * 128 : (ks + 1) * 128]
                            for qn in range(Q // 512):
                                nc.tensor.matmul(
                                    sps[:, qn * 512 : (qn + 1) * 512],
                                    lhsT,
                                    QT[hp : hp + 64, eo, qn * 512 : (qn + 1) * 512],
                                    start=True,
                                    stop=True,
                                )
                            eb = ebuf.tile([128, Q], BF16)
                            nc.scalar.activation(
                                out=eb,
                                in_=sps,
                                func=mybir.ActivationFunctionType.Exp,
                                scale=0.125,
                            )
                            pb = pbuf.tile([128, Q], BF16)
                            nc.vector.tensor_tensor(
                                out=pb,
                                in0=eb,
                                in1=MB[:, ks, :],
                                op=mybir.AluOpType.mult,
                            )
                            for qn in range(Q // 512):
                                nc.tensor.matmul(
                                    pv[:, qn * 512 : (qn + 1) * 512],
                                    V[:, ks, h, :],
                                    pb[:, qn * 512 : (qn + 1) * 512],
                                    start=(ks == 0),
                                    stop=(ks == 15),
                                )
                        ob = oab.tile([64, Q], F32)
                        nc.vector.tensor_copy(out=ob, in_=pv[0:64, :])
                        nc.sync.dma_start(
                            out=oa_dram[h * 64 : (h + 1) * 64, :], in_=ob
                        )
                        dt = dtmp.tile([1, Q], F32)
                        nc.vector.tensor_copy(out=dt, in_=pv[64:65, :])
                        nc.sync.dma_start(out=den_dram[h : h + 1, :], in_=dt)

        if phases == "p0p1":
            return
        # QT/KT/V dead after P1 -- release their SBUF before P2
        pctx.close()

        # ---- P2: normalize + output projection ----
        with (
            tc.tile_pool(name="p2", bufs=1) as p2,
            tc.tile_pool(name="rb", bufs=2) as rb,
            tc.tile_pool(name="ybuf", bufs=2) as ybuf,
            tc.tile_pool(name="psY", bufs=4, space="PSUM") as psY,
        ):
            WO = p2.tile([128, 8, D], F32R)
            nc.sync.dma_start(out=WO, in_=_r(wo3))
            OA = p2.tile([128, 8, Q], F32R)
            oa_dram3 = oa_dram[:, :].rearrange("(jo ji) q -> ji jo q", ji=128)
            for j in range(8):
                nc.sync.dma_start(out=OA[:, j, :], in_=_r(oa_dram3[:, j, :]))

            # 1/den via ACT: exp(-ln(den)) (same table set as Exp)
            den_sb = p2.tile([H, Q], F32)
            nc.sync.dma_start(out=den_sb, in_=den_dram[:, :])
            den_r = p2.tile([H, Q], F32)
            nc.scalar.activation(
                out=den_r, in_=den_sb, func=mybir.ActivationFunctionType.Ln
            )
            nc.scalar.activation(
                out=den_r,
                in_=den_r,
                func=mybir.ActivationFunctionType.Exp,
                scale=-1.0,
            )
            nc.sync.dma_start(out=denr_dram[:, :], in_=den_r)

            for j in range(8):
                # R[p, q]: rows 0:64 = 1/den[2j], rows 64:128 = 1/den[2j+1]
                R = rb.tile([128, Q], F32)
                for half in range(2):
                    nc.sync.dma_start(
                        out=R[half * 64 : (half + 1) * 64, :],
                        in_=denr_dram[2 * j + half : 2 * j + half + 1, :].to_broadcast(
                            (64, Q)
                        ),
                    )
                nc.vector.tensor_tensor(
                    out=OA[:, j, :],
                    in0=OA[:, j, :].bitcast(F32),
                    in1=R,
                    op=mybir.AluOpType.mult,
                )

            # y[q, eo] = sum_e OA[e, q] * woT[e, eo]
            for qt in range(Q // 128):
                ps = psY.tile([128, D], F32)
                for j in range(8):
                    for en in range(2):
                        nc.tensor.matmul(
                            ps[:, en * 512 : (en + 1) * 512],
                            OA[:, j, qt * 128 : (qt + 1) * 128],
                            WO[:, j, en * 512 : (en + 1) * 512],
                            start=(j == 0),
                            stop=(j == 7),
                        )
                yb = ybuf.tile([128, D], F32)
                nc.vector.tensor_copy(out=yb, in_=ps)
                nc.sync.dma_start(out=y[qt * 128 : (qt + 1) * 128, :], in_=yb)


def _get_nc(phases="all"):
    if phases in _NC_CACHE:
        return _NC_CACHE[phases]
    nc = bacc.Bacc("TRN2", target_bir_lowering=False)
    t_in = {
        "qT": nc.dram_tensor("qT", [D, Q], F32, kind="ExternalInput"),
        "kT": nc.dram_tensor("kT", [D, S], F32, kind="ExternalInput"),
        "vT": nc.dram_tensor("vT", [D, S], F32, kind="ExternalInput"),
        "mT": nc.dram_tensor("mT", [S, Q], I32, kind="ExternalInput"),
        "wqT": nc.dram_tensor("wqT", [D, D], F32, kind="ExternalInput"),
        "wkT": nc.dram_tensor("wkT", [D, D], F32, kind="ExternalInput"),
        "wvT": nc.dram_tensor("wvT", [D, D], F32, kind="ExternalInput"),
        "woT": nc.dram_tensor("woT", [D, D], F32, kind="ExternalInput"),
    }
    t_out = {"y": nc.dram_tensor("y", [Q, D], F32, kind="ExternalOutput")}
    with tile.TileContext(nc) as tc:
        _build_kernel(tc, t_in, t_out, phases=phases)
    nc.compile()
    _NC_CACHE[phases] = nc
    return nc


def _in_maps(inputs):
    q = np.asarray(inputs["query"], np.float32)
    k = np.asarray(inputs["key"], np.float32)
    v = np.asarray(inputs["value"], np.float32)
    mask = np.asarray(inputs["mask"], np.int32)
    wqT = np.ascontiguousarray(np.asarray(inputs["wq"], np.float32).T)
    wkT = np.ascontiguousarray(np.asarray(inputs["wk"], np.float32).T)
    wvT = np.ascontiguousarray(np.asarray(inputs["wv"], np.float32).T)
    woT = np.ascontiguousarray(np.asarray(inputs["w_out"], np.float32).T)
    maps = []
    for c in range(NCORES):
        b, qh = c // 2, c % 2
        sl = slice(qh * Q, (qh + 1) * Q)
        maps.append(
            {
                "qT": np.ascontiguousarray(q[b].T[:, sl]),
                "kT": np.ascontiguousarray(k[b].T),
                "vT": np.ascontiguousarray(v[b].T),
                "mT": np.ascontiguousarray(mask[b].T[:, sl]),
                "wqT": wqT,
                "wkT": wkT,
                "wvT": wvT,
                "woT": woT,
            }
        )
    return maps


def _gather(res):
    outs = [res.results[c]["y"] for c in range(NCORES)]
    return np.stack(
        [np.concatenate([outs[2 * b], outs[2 * b + 1]], axis=0) for b in range(B)]
    )


def kernel(**inputs) -> np.ndarray:
    nc = _get_nc()
    res = run_bass_kernel_spmd(nc, _in_maps(inputs), core_ids=list(range(NCORES)))
    return _gather(res)


def kernel_traced(**inputs):
    """Like kernel() but with NTFF tracing; returns (output, BassKernelResults)."""
    nc = _get_nc()
    res = run_bass_kernel_spmd(
        nc, _in_maps(inputs), core_ids=list(range(NCORES)), trace=True
    )
    return _gather(res), res
